# revision 11
# baseline (speedup 1.0000x reference)
"""Trainium2 Bass kernel for a MountainCar dynamics-model RNN.

Model (per batch element, T=500 steps, H=40):
    xn      = (x - MEAN_IN) / STD_IN                       # [T, 3]
    s2_{t+1} = tanh(xn_t[:2] @ Ks + xn_t[2:] @ Ka + bias + s2_t @ Kh)
    out     = clip((s2_T @ dense_w + dense_b) * STD_OUT + MEAN_OUT)

Device mapping (8 NeuronCores, batch-parallel, 4096 batch each):
  - state layout: [H=40 partitions, batch on free dim], two partition
    "homes" {0-39, 64-103} so one ScalarE tanh op covers 80 useful lanes.
  - per step: 8 x-projection matmuls (K=3, all concurrent via PE array
    tiling) + 8 recurrence matmuls (K=40, 2x2 array tiling) accumulate
    into PSUM; 2 tanh ops (free-dim halves) write the next state.
  - input normalization, weight folding, and the x time-major transpose
    are done host-side in numpy; per-step x arrives via one small DMA.
"""

import os
import sys

sys.path.insert(0, "/opt/trn_rl_repo")

import numpy as np

# ---------------------------------------------------------------- constants
B, T_FULL, F, H = 32768, 500, 3, 40
NCORES = 8
BS = B // NCORES          # 4096 batch per core
G = 512                   # batch per matmul group (one PSUM bank)
NJ = 4                    # free-dim group slots per home
NH = 2                    # partition homes (0-39, 64-103)

MEAN_IN = np.array([-0.3, 0.0, 0.0], np.float32)
STD_IN = np.array([0.9, 0.07, 1.0], np.float32)
MEAN_OUT = np.array([-0.3, 0.0], np.float32)
STD_OUT = np.array([0.9, 0.07], np.float32)
MIN_POS, MAX_POS, MAX_SPEED = -1.2, 0.6, 0.07


# ---------------------------------------------------------------- host prep
def _host_weights(kernel_state, kernel_hidden, kernel_action, bias,
                  dense_w, dense_b):
    """Fold normalization into the weights; build partition-home layouts."""
    w3 = np.vstack([kernel_state, kernel_action]).astype(np.float32)  # [3, 40]
    w3p = w3 / STD_IN[:, None]                                        # [3, 40]
    cp = (bias[0] - (MEAN_IN / STD_IN) @ w3).astype(np.float32)       # [40]
    kh = kernel_hidden.astype(np.float32)                             # [40, 40]

    kh2 = np.zeros((104, 40), np.float32)
    kh2[0:40] = kh
    kh2[64:104] = kh

    w34 = np.zeros((99, 40), np.float32)
    for j in range(NJ):
        w34[32 * j:32 * j + 3] = w3p

    c2 = np.zeros((104, 1), np.float32)
    c2[0:40, 0] = cp
    c2[64:104, 0] = cp

    dwp = (dense_w * STD_OUT[None, :]).astype(np.float32)             # [40, 2]
    dbp = (dense_b * STD_OUT + MEAN_OUT).astype(np.float32)           # [2]
    dw2 = np.zeros((104, 2), np.float32)
    dw2[0:40] = dwp
    dw2[64:104] = dwp

    db2 = np.zeros((104, 1), np.float32)
    lo2 = np.zeros((104, 1), np.float32)
    hi2 = np.zeros((104, 1), np.float32)
    for h in range(NH):
        db2[64 * h + 0, 0], db2[64 * h + 1, 0] = dbp[0], dbp[1]
        lo2[64 * h + 0, 0], lo2[64 * h + 1, 0] = MIN_POS, -MAX_SPEED
        hi2[64 * h + 0, 0], hi2[64 * h + 1, 0] = MAX_POS, MAX_SPEED
    return dict(kh2=kh2, w34=w34, c2=c2, dw2=dw2, db2=db2, lo2=lo2, hi2=hi2)


def _host_x_shard(x_core, t_steps):
    """[BS, T, 3] -> [T, NJ, 3, NH*G] time-major, grouped for the SBUF layout.

    Group (h, j) covers batch range [2048*h + 512*j, +512); on SBUF the x
    for that group lands at partitions 32j..32j+2, free 512h..512h+512.
    """
    xs = x_core.reshape(NH, NJ, G, t_steps, F)
    xt = xs.transpose(3, 1, 4, 0, 2).reshape(t_steps, NJ, F, NH * G)
    return np.ascontiguousarray(xt, dtype=np.float32)


# ---------------------------------------------------------------- bass prog
def build_program(t_steps=T_FULL, repeat=1):
    import concourse.bass as bass
    import concourse.tile as tile
    from concourse import bacc, mybir
    from concourse._compat import with_exitstack
    from contextlib import ExitStack

    f32 = mybir.dt.float32
    nc = bacc.Bacc("TRN2", target_bir_lowering=False, debug=False,
                   enable_asserts=True, num_devices=NCORES)

    ins = {}
    for name, shape in [("xt", [t_steps, NJ, F, NH * G]),
                        ("kh2", [104, 40]), ("w34", [99, 40]),
                        ("c2", [104, 1]), ("dw2", [104, 2]),
                        ("db2", [104, 1]), ("lo2", [104, 1]),
                        ("hi2", [104, 1])]:
        ins[name] = nc.dram_tensor(name, shape, f32, kind="ExternalInput").ap()
    out_d = nc.dram_tensor("out", [2, BS], f32, kind="ExternalOutput").ap()

    Tanh = mybir.ActivationFunctionType.Tanh
    Alu = mybir.AluOpType

    @with_exitstack
    def body(ctx: ExitStack, tc: tile.TileContext):
        nc = tc.nc
        singles = ctx.enter_context(tc.tile_pool(name="singles", bufs=1))
        xpool = ctx.enter_context(tc.tile_pool(name="xp", bufs=6))
        spool = ctx.enter_context(tc.tile_pool(name="state", bufs=1))
        pspool = ctx.enter_context(tc.tile_pool(name="ps", bufs=1, space="PSUM"))

        kh_t = singles.tile([104, 40], f32)
        nc.sync.dma_start(out=kh_t[:], in_=ins["kh2"])
        w3_t = singles.tile([99, 40], f32)
        nc.sync.dma_start(out=w3_t[:], in_=ins["w34"])
        c_t = singles.tile([104, 1], f32)
        nc.sync.dma_start(out=c_t[:], in_=ins["c2"])
        dw_t = singles.tile([104, 2], f32)
        nc.sync.dma_start(out=dw_t[:], in_=ins["dw2"])
        db_t = singles.tile([104, 1], f32)
        nc.sync.dma_start(out=db_t[:], in_=ins["db2"])
        lo_t = singles.tile([104, 1], f32)
        nc.sync.dma_start(out=lo_t[:], in_=ins["lo2"])
        hi_t = singles.tile([104, 1], f32)
        nc.sync.dma_start(out=hi_t[:], in_=ins["hi2"])

        S = [spool.tile([104, NH * G * NJ // 2], f32, tag=f"s{i}",
                        name=f"s{i}") for i in range(2)]
        PS = [pspool.tile([128, 2048], f32, tag=f"ps{i}", name=f"ps{i}")
              for i in range(2)]
        nc.vector.memset(S[0][:], 0.0)
        # tanh reads the full 0-103 partition span; rows 40-63 are never
        # written by any matmul, so give them defined values once. (PSUM
        # engine accesses need a 32-aligned partition base, so start at 32;
        # rows 32-39 are overwritten by the first matmuls anyway.)
        nc.vector.memset(PS[0][32:64, :], 0.0)
        nc.vector.memset(PS[1][32:64, :], 0.0)

        def step(t):
            ps = PS[t % 2]
            cur, nxt = S[t % 2], S[(t + 1) % 2]
            xb = xpool.tile([99, NH * G], f32, name="xb", tag="xb")
            for j in range(NJ):
                nc.sync.dma_start(out=xb[32 * j:32 * j + 3, :],
                                  in_=ins["xt"][t, j])
            # x-projection: psum[h', b] = sum_f w3p[f, h'] * x[f, b]
            for j in range(NJ):
                for h in range(NH):
                    nc.tensor.matmul(
                        ps[64 * h:64 * h + 40, G * j:G * (j + 1)],
                        lhsT=w3_t[32 * j:32 * j + 3, :],
                        rhs=xb[32 * j:32 * j + 3, G * h:G * (h + 1)],
                        start=True, stop=False,
                        skip_group_check=True,
                        tile_position=(32 * j, 64 * h))
            # recurrence: accumulate s2 @ Kh into the same psum regions
            for j in range(NJ):
                for h in range(NH):
                    nc.tensor.matmul(
                        ps[64 * h:64 * h + 40, G * j:G * (j + 1)],
                        lhsT=kh_t[64 * h:64 * h + 40, :],
                        rhs=cur[64 * h:64 * h + 40, G * j:G * (j + 1)],
                        start=False, stop=True,
                        skip_group_check=True,
                        tile_position=(64 * h, 64 * h))
            # s2' = tanh(psum + c'); two free-dim halves so the next step's
            # matmuls can start as soon as the first half lands.
            for half in range(2):
                fs = slice(1024 * half, 1024 * (half + 1))
                nc.scalar.activation(nxt[0:104, fs], ps[0:104, fs], Tanh,
                                     bias=c_t[0:104, 0:1])

        if repeat > 1:
            # timing-only amplifier: rerun the whole recurrence in a
            # hardware loop inside one NEFF so device time dominates host
            # dispatch noise. Output is only valid for repeat=1.
            with tc.For_i(0, repeat, 1):
                for t in range(t_steps):
                    step(t)
        else:
            for t in range(t_steps):
                step(t)

        # endgame: out = clip((s2 @ dw') + db')
        sf = S[t_steps % 2]
        pse = PS[t_steps % 2]
        for j in range(NJ):
            for h in range(NH):
                nc.tensor.matmul(
                    pse[64 * h:64 * h + 2, G * j:G * (j + 1)],
                    lhsT=dw_t[64 * h:64 * h + 40, :],
                    rhs=sf[64 * h:64 * h + 40, G * j:G * (j + 1)],
                    start=True, stop=True,
                    skip_group_check=True,
                    tile_position=(64 * h, 64 * h))
        ob = singles.tile([104, 2048], f32)
        for h in range(NH):
            rs = slice(64 * h, 64 * h + 2)
            nc.vector.tensor_scalar(ob[rs, :], pse[rs, :],
                                    scalar1=db_t[rs, 0:1],
                                    scalar2=hi_t[rs, 0:1],
                                    op0=Alu.add, op1=Alu.min)
            nc.vector.tensor_scalar(ob[rs, :], ob[rs, :],
                                    scalar1=lo_t[rs, 0:1], scalar2=None,
                                    op0=Alu.max)
            nc.sync.dma_start(out=out_d[:, 2048 * h:2048 * (h + 1)],
                              in_=ob[rs, :])

    import concourse.tile as tile_mod
    with tile_mod.TileContext(nc) as tc:
        body(tc)
    nc.compile()
    return nc


# ---------------------------------------------------------------- execution
def _make_in_maps(x, weights, t_steps):
    in_maps = []
    for c in range(NCORES):
        m = dict(weights)
        m["xt"] = _host_x_shard(
            np.asarray(x[c * BS:(c + 1) * BS, :t_steps], dtype=np.float32),
            t_steps)
        in_maps.append(m)
    return in_maps


def run(x, kernel_state, kernel_hidden, kernel_action, bias, dense_w,
        dense_b, t_steps=T_FULL, trace=False, nc=None):
    from concourse.bass_utils import run_bass_kernel_spmd
    if nc is None:
        nc = build_program(t_steps)
    weights = _host_weights(kernel_state, kernel_hidden, kernel_action,
                            bias, dense_w, dense_b)
    in_maps = _make_in_maps(x, weights, t_steps)
    res = run_bass_kernel_spmd(nc, in_maps, core_ids=list(range(NCORES)),
                               trace=trace)
    outs = [res.results[c]["out"].T for c in range(NCORES)]  # [BS, 2] each
    return np.concatenate(outs, axis=0), res


def kernel(x, kernel_state, kernel_hidden, kernel_action, bias, dense_w,
           dense_b):
    out, _ = run(np.asarray(x), np.asarray(kernel_state),
                 np.asarray(kernel_hidden), np.asarray(kernel_action),
                 np.asarray(bias), np.asarray(dense_w), np.asarray(dense_b))
    return out
